# revision 1
# baseline (speedup 1.0000x reference)
"""DenoiseNet loss kernel for 8 Trainium2 NeuronCores.

Strategy: pure data parallel over the batch (4 batches/core). PointNet MLP in
fp16 (fp32 PSUM accumulate), exact global BatchNorm via per-layer AllReduce of
(sum, sumsq). KNN argmin via hi/lo-fp16-split matmul of 2q.r-|r|^2 (accurate
to ~1e-5), extraction of the matched |v-r*|^2 by is_equal one-hot + fused
multiply-reduce. Loss partials per core summed on host.
"""
import numpy as np

import concourse.bass as bass
import concourse.mybir as mybir
import concourse.tile as tile
from concourse import bacc
from concourse.bass_utils import run_bass_kernel_spmd

dt = mybir.dt
F32 = dt.float32
F16 = dt.float16
AF = mybir.ActivationFunctionType
OP = mybir.AluOpType
AX = mybir.AxisListType

B, N, NCORES = 32, 1000, 8
BL = B // NCORES            # 4 batches per core
PTS = BL * N                # 4000 points per core
NITER = 4
NPTS_GLOBAL = B * N         # 32000 (BN population)
EPS = 1e-5
NOISE_DECAY = 4.0
QT = 125                    # q tile (8 per batch)
RP = 1024                   # padded ref points (24 sentinels)
PT = 500                    # pts tile for MLP (8 tiles)
NPT = PTS // PT

# (C_in, C_out, has_bn) per layer
LAYERS = [(3, 64, 1), (64, 128, 1), (128, 256, 1), (256, 512, 1),
          (512, 1024, 1), (1024, 512, 1), (512, 256, 1), (256, 3, 0)]
NCI = [max(1, ci // 128) for ci, co, _ in LAYERS]
NCO = [max(1, (co + 127) // 128) for ci, co, _ in LAYERS]

RG = [list(range(NCORES))]

_NC_CACHE = {}


def _build(niter=NITER, nlayers=8, do_knn=True, do_ar=True):
    nc = bacc.Bacc(None, target_bir_lowering=False, debug=False)

    x0t_d = nc.dram_tensor("x0t", [3, PTS], F32, kind="ExternalInput")
    cneg_d = nc.dram_tensor("cneg", [2, PTS], F16, kind="ExternalInput")
    cpos_d = nc.dram_tensor("cpos", [2, PTS], F16, kind="ExternalInput")
    sw_d = nc.dram_tensor("sw", [128, 32], F32, kind="ExternalInput")
    sws3_d = nc.dram_tensor("sws3", [3, PTS], F16, kind="ExternalInput")
    db3_d = nc.dram_tensor("db3t", [3, NITER], F32, kind="ExternalInput")
    r_d = [[nc.dram_tensor(f"rknn_{i}_{b}", [13, RP], F16, kind="ExternalInput")
            for b in range(BL)] for i in range(NITER)]
    w_d = [[nc.dram_tensor(f"w_{i}_{l}", list(LAYERS[l][:2]), F16,
                           kind="ExternalInput") for l in range(8)]
           for i in range(NITER)]
    gs_d = [[nc.dram_tensor(f"gs_{i}_{l}", [128, 3, NCO[l]], F32,
                            kind="ExternalInput") for l in range(7)]
            for i in range(NITER)]
    loss_d = nc.dram_tensor("loss_part", [128, 1], F32, kind="ExternalOutput")
    loss3_d = nc.dram_tensor("loss_part3", [3, 1], F32, kind="ExternalOutput")

    with tile.TileContext(nc) as tc:
        with (
            tc.tile_pool(name="sb", bufs=1) as sb,
            tc.tile_pool(name="ps", bufs=4, space="PSUM") as ps,
            tc.tile_pool(name="psk", bufs=1, space="PSUM") as psk,
            tc.tile_pool(name="dram", bufs=2, space="DRAM") as dram,
        ):
            # ---------- persistent setup ----------
            sw_sb = sb.tile([128, 32], F32, tag="sw")
            nc.gpsimd.dma_start(sw_sb[:], sw_d[:])
            sws3 = sb.tile([3, PTS], F16, tag="sws3")
            nc.gpsimd.dma_start(sws3[:], sws3_d[:])
            db3_sb = sb.tile([3, NITER], F32, tag="db3")
            nc.gpsimd.dma_start(db3_sb[:], db3_d[:])

            x_cur = sb.tile([3, PTS], F32, tag="xA")
            nc.gpsimd.dma_start(x_cur[:], x0t_d[:])

            Ld = sb.tile([11, PTS], F16, tag="Ld")
            nc.gpsimd.dma_start(Ld[9:11, :], cneg_d[:])
            Le = sb.tile([11, PTS], F16, tag="Le")
            nc.gpsimd.dma_start(Le[9:11, :], cpos_d[:])

            eps_sb = sb.tile([128, 1], F32, tag="epsc")
            nc.vector.memset(eps_sb[:], float(EPS))
            licol = sb.tile([128, NITER], F32, tag="licol")
            nc.vector.memset(licol[:], 0.0)
            a3col = sb.tile([3, NITER], F32, tag="a3col")
            nc.vector.memset(a3col[:], 0.0)

            for it in range(niter):
                r_sb = []
                for b in range(BL):
                    rt_ = sb.tile([13, RP], F16, tag=f"r{b}")
                    nc.gpsimd.dma_start(rt_[:], r_d[it][b][:])
                    r_sb.append(rt_)

                # ---------- phase 1: Ld build + d' matmuls + row max ----------
                if not do_knn:
                    m_all = None
                # Ld rows: [qh(3), ql(3), qh(3), -1, -1] with qh=f16(2x),
                # ql=f16(2x-qh). Engine writes must start at a 32-aligned
                # partition, so rows 3..8 are assembled by SBUF-SBUF DMA.
                if do_knn:
                  nc.vector.tensor_scalar_mul(Ld[0:3, :], x_cur[:], 2.0)
                  ql16 = sb.tile([3, PTS], F16, tag="ql16")
                  nc.vector.scalar_tensor_tensor(ql16[:], x_cur[:], 2.0,
                                               Ld[0:3, :], OP.mult,
                                               OP.subtract)
                  nc.gpsimd.dma_start(Ld[3:6, :], ql16[:])
                  nc.gpsimd.dma_start(Ld[6:9, :], Ld[0:3, :])

                  m_all = sb.tile([128, 32], F32, tag="m_all")
                  for b in range(BL):
                    for qt in range(8):
                        col = b * 8 + qt
                        qsl = slice(b * N + qt * QT, b * N + (qt + 1) * QT)
                        kpd = psk.tile([QT, 2, 512], F32, tag="kpd")
                        for rt in range(2):
                            nc.tensor.matmul(
                                kpd[:, rt, :], Ld[0:11, qsl],
                                r_sb[b][0:11, rt * 512:(rt + 1) * 512],
                                start=True, stop=True)
                        nc.vector.tensor_reduce(m_all[0:QT, col:col + 1],
                                                kpd[:], AX.XY, OP.max)

                # ---------- phase 2: MLP ----------
                xf16 = sb.tile([3, PTS], F16, tag="xf")
                nc.vector.tensor_copy(xf16[:], x_cur[:])
                rhs = [xf16]
                amag_prev = None  # [128, nco_prev] ap for weight folding

                for l in range(nlayers):
                    cin, cout, has_bn = LAYERS[l]
                    nci, nco = NCI[l], NCO[l]
                    CIP = min(128, cin)  # partitions per ci chunk

                    if has_bn:
                        gs = sb.tile([128, 3, nco], F32, tag="gs", bufs=2)
                        nc.gpsimd.dma_start(gs[:], gs_d[it][l][:])
                        sums = sb.tile([128, nco, NPT], F32, tag="sums", bufs=2)
                        statsr = sb.tile([128, 2, nco], F32, tag="statsr",
                                         bufs=2)
                        zt = [sb.tile([128, PTS], F16, tag=f"z{l % 2}_{co}",
                                      name=f"z_{it}_{l}_{co}")
                              for co in range(nco)]
                        preds = None
                    else:
                        preds = sb.tile([3, PTS], F32, tag="scr3")
                        zt = None

                    for co in range(nco):
                        CO = min(128, cout - co * 128)
                        # JIT-load this co-column of weights (all ci chunks)
                        wt = []
                        for ci in range(nci):
                            w = sb.tile([CIP, CO], F16, tag=f"wc{ci}", bufs=2)
                            nc.gpsimd.dma_start(
                                w[:],
                                w_d[it][l][ci * 128:ci * 128 + CIP,
                                           co * 128:co * 128 + CO])
                            if amag_prev is not None:
                                nc.vector.tensor_scalar_mul(
                                    w[:], w[:], amag_prev[0:CIP, ci:ci + 1])
                            wt.append(w)
                        for pt in range(NPT):
                            ptsl = slice(pt * PT, (pt + 1) * PT)
                            zp = ps.tile([128, PT], F32, tag="zp")
                            for ci in range(nci):
                                nc.tensor.matmul(
                                    zp[0:CO, :], wt[ci][:],
                                    rhs[ci][0:CIP, ptsl],
                                    start=(ci == 0), stop=(ci == nci - 1))
                            if has_bn:
                                nc.scalar.activation(
                                    zt[co][0:CO, ptsl], zp[0:CO, :], AF.Copy,
                                    accum_out=sums[0:CO, co, pt:pt + 1])
                            else:
                                nc.scalar.activation(
                                    preds[:, ptsl], zp[0:3, :], AF.Tanh,
                                    bias=db3_sb[:, it:it + 1])

                        if has_bn:
                            jk16 = sb.tile([128, PTS], F16, tag="junk16")
                            nc.scalar.activation(
                                jk16[0:CO, :], zt[co][0:CO, :], AF.Square,
                                accum_out=statsr[0:CO, 1, co:co + 1])

                    if not has_bn:
                        break

                    nc.vector.tensor_reduce(statsr[:, 0, :], sums[:], AX.X,
                                            OP.add)
                    arin = dram.tile([128, 2, nco], F32, tag="arin")
                    arout = dram.tile([128, 2, nco], F32, tag="arout")
                    nc.gpsimd.dma_start(arin[:], statsr[:])
                    if do_ar:
                        nc.gpsimd.collective_compute(
                            "AllReduce", OP.add, replica_groups=RG,
                            ins=[arin.opt()], outs=[arout.opt()])
                    else:
                        nc.gpsimd.dma_start(arout[:], arin[:])
                    statsg = sb.tile([128, 2, nco], F32, tag="statsg", bufs=2)
                    nc.gpsimd.dma_start(statsg[:], arout[:])

                    # affine: a = g*rsqrt(var+eps); ct = (h - mean*a)/|a|
                    af = sb.tile([128, 6, nco], F32, tag="af", bufs=2)
                    inv_n = 1.0 / NPTS_GLOBAL
                    nc.vector.tensor_scalar_mul(af[:, 0, :], statsg[:, 0, :],
                                                inv_n)
                    nc.vector.tensor_scalar_mul(af[:, 1, :], statsg[:, 1, :],
                                                inv_n)
                    nc.vector.tensor_tensor(out=af[:, 2, :], in0=af[:, 0, :],
                                            in1=af[:, 0, :], op=OP.mult)
                    nc.vector.tensor_tensor(out=af[:, 1, :], in0=af[:, 1, :],
                                            in1=af[:, 2, :], op=OP.subtract)
                    nc.scalar.activation(af[:, 2, :], af[:, 1, :], AF.Sqrt,
                                         bias=eps_sb[:])
                    nc.vector.reciprocal(af[:, 3, :], af[:, 2, :])
                    nc.vector.tensor_tensor(out=af[:, 4, :], in0=gs[:, 0, :],
                                            in1=af[:, 3, :], op=OP.mult)
                    nc.scalar.activation(af[:, 5, :], af[:, 4, :], AF.Abs)
                    nc.vector.tensor_tensor(out=af[:, 2, :], in0=af[:, 0, :],
                                            in1=af[:, 4, :], op=OP.mult)
                    nc.vector.tensor_tensor(out=af[:, 0, :], in0=gs[:, 1, :],
                                            in1=af[:, 2, :], op=OP.subtract)
                    nc.vector.reciprocal(af[:, 1, :], af[:, 5, :])
                    nc.vector.tensor_tensor(out=af[:, 0, :], in0=af[:, 0, :],
                                            in1=af[:, 1, :], op=OP.mult)

                    for co in range(nco):
                        CO = min(128, cout - co * 128)
                        nc.vector.tensor_scalar(
                            zt[co][0:CO, :], zt[co][0:CO, :],
                            gs[0:CO, 2, co:co + 1], af[0:CO, 0, co:co + 1],
                            OP.mult, OP.add)
                        nc.vector.tensor_scalar_max(
                            zt[co][0:CO, :], zt[co][0:CO, :], 0.0)
                    amag_prev = af[:, 5, :]
                    rhs = zt

                # ---------- phase 3: X <- X + pred (in place) ----------
                if nlayers < 8:
                    continue
                nc.vector.tensor_tensor(out=x_cur[:], in0=x_cur[:],
                                        in1=preds[:], op=OP.add)
                if not do_knn:
                    continue

                # ---------- phase 4: Le build + e matmuls + extraction ------
                nc.vector.tensor_scalar_mul(Le[0:3, :], x_cur[:], -2.0)
                lel16 = sb.tile([3, PTS], F16, tag="ql16")
                nc.vector.scalar_tensor_tensor(lel16[:], x_cur[:], -2.0,
                                               Le[0:3, :], OP.mult,
                                               OP.subtract)
                nc.gpsimd.dma_start(Le[3:6, :], lel16[:])
                nc.gpsimd.dma_start(Le[6:9, :], Le[0:3, :])

                S = sb.tile([128, 32], F32, tag="S", bufs=2)
                nc.vector.memset(S[:], 0.0)
                for b in range(BL):
                    for qt in range(8):
                        col = b * 8 + qt
                        qsl = slice(b * N + qt * QT, b * N + (qt + 1) * QT)
                        kpd = psk.tile([QT, 2, 512], F32, tag="kpd")
                        for rt in range(2):
                            nc.tensor.matmul(
                                kpd[:, rt, :], Ld[0:11, qsl],
                                r_sb[b][0:11, rt * 512:(rt + 1) * 512],
                                start=True, stop=True)
                        kpe = psk.tile([QT, 2, 512], F32, tag="kpe")
                        for rt in range(2):
                            nc.tensor.matmul(
                                kpe[:, rt, :], Le[0:11, qsl],
                                r_sb[b][0:11, rt * 512:(rt + 1) * 512],
                                start=True, stop=True)
                        e16 = sb.tile([QT, 2, 512], F16, tag="scrap", bufs=3)
                        nc.scalar.activation(e16[:], kpe[:], AF.Copy)
                        jk1 = sb.tile([QT, 2, 512], F16, tag="scrap", bufs=3)
                        nc.vector.scalar_tensor_tensor(
                            jk1[:], kpd[:], m_all[0:QT, col:col + 1], e16[:],
                            OP.is_equal, OP.mult,
                            accum_out=S[0:QT, col:col + 1])

                # term2: sum_q sw_q |v_q|^2 via (v*sqrt(sw))^2
                wv = sb.tile([3, PTS], F16, tag="ql16")
                nc.vector.tensor_tensor(out=wv[:], in0=x_cur[:], in1=sws3[:],
                                        op=OP.mult)
                jk3 = sb.tile([3, PTS], F16, tag="junk16")
                nc.vector.scalar_tensor_tensor(
                    jk3[:], wv[:], 1.0, wv[:], OP.mult, OP.mult,
                    accum_out=a3col[0:3, it:it + 1])

                jk2 = sb.tile([128, 32], F32, tag="jk2", bufs=2)
                nc.vector.scalar_tensor_tensor(
                    jk2[:], S[:], 1.0, sw_sb[:], OP.mult, OP.mult,
                    accum_out=licol[:, it:it + 1])

            lacc = sb.tile([128, 1], F32, tag="laccA")
            nc.vector.tensor_reduce(lacc[:], licol[:], AX.X, OP.add)
            acc3f = sb.tile([3, 1], F32, tag="acc3A")
            nc.vector.tensor_reduce(acc3f[:], a3col[0:3, :], AX.X, OP.add)
            nc.gpsimd.dma_start(loss_d[:], lacc[:])
            nc.gpsimd.dma_start(loss3_d[:], acc3f[:])
    nc.compile()
    return nc


def _host_prep(inputs):
    """Build per-core input maps."""
    f32 = np.float32
    noisy = np.asarray(inputs["pcl_noisy"], f32)
    clean = np.asarray(inputs["pcl_clean"], f32)
    seeds = np.asarray(inputs["pcl_seeds"], f32)
    std = np.asarray(inputs["pcl_std"], f32)
    noise = np.asarray(inputs["noise"], f32)

    pn = noisy - seeds
    pc = clean - seeds
    sdist = np.sum(pn.astype(np.float64) ** 2, -1, keepdims=True)
    max_sq = sdist[:, -1:, :]
    sw = np.exp(-sdist * 9.0 / max_sq)[..., 0]
    sw = (sw / sw.sum(1, keepdims=True))  # [B, N] float64

    tgts = []
    cur = std.copy()
    for i in range(NITER):
        if i < NITER - 1:
            cur = cur / NOISE_DECAY
            tgts.append(pc + noise[i] * cur[:, None, None])
        else:
            tgts.append(pc.copy())

    sent = np.full((RP - N, 3), 100.0, np.float64)

    shared = {}
    for i in range(NITER):
        for l in range(8):
            key = f'ew{l+1}' if l < 5 else f'dw{l-4}'
            shared[f"w_{i}_{l}"] = np.asarray(inputs[key], f32)[i].astype(
                np.float16)
        for l in range(7):
            nco = NCO[l]
            cout = LAYERS[l][1]
            gk = f'eg{l+1}' if l < 5 else f'dg{l-4}'
            hk = f'eh{l+1}' if l < 5 else f'dh{l-4}'
            g = np.asarray(inputs[gk], f32)[i]
            h = np.asarray(inputs[hk], f32)[i]
            arr = np.zeros((128, 3, nco), f32)
            gp = np.zeros(nco * 128, f32); gp[:cout] = g
            hp = np.zeros(nco * 128, f32); hp[:cout] = h
            sp = np.where(gp >= 0, 1.0, -1.0).astype(f32)
            arr[:, 0, :] = gp.reshape(nco, 128).T
            arr[:, 1, :] = hp.reshape(nco, 128).T
            arr[:, 2, :] = sp.reshape(nco, 128).T
            shared[f"gs_{i}_{l}"] = arr
    shared["db3t"] = np.ascontiguousarray(np.asarray(inputs["db3"], f32).T)
    shared["cneg"] = np.full((2, PTS), -1.0, np.float16)
    shared["cpos"] = np.full((2, PTS), 1.0, np.float16)

    in_maps = []
    for c in range(NCORES):
        bs = slice(c * BL, (c + 1) * BL)
        m = dict(shared)
        m["x0t"] = np.ascontiguousarray(
            pn[bs].transpose(2, 0, 1).reshape(3, PTS))
        swc = np.zeros((128, 32), f32)
        for b in range(BL):
            for qt in range(8):
                swc[0:QT, b * 8 + qt] = sw[c * BL + b,
                                           qt * QT:(qt + 1) * QT].astype(f32)
        m["sw"] = swc
        m["sws3"] = np.broadcast_to(
            np.sqrt(sw[bs]).reshape(1, PTS), (3, PTS)).astype(np.float16)
        for i in range(NITER):
            for b in range(BL):
                coords = np.concatenate(
                    [tgts[i][c * BL + b].astype(np.float64), sent], 0)
                rh = coords.astype(np.float16)
                rl = (coords - rh.astype(np.float64)).astype(np.float16)
                rsq = (coords ** 2).sum(1)
                rsqh = rsq.astype(np.float16)
                rsql = (rsq - rsqh.astype(np.float64)).astype(np.float16)
                R = np.empty((13, RP), np.float16)
                R[0:3] = rh.T
                R[3:6] = rh.T
                R[6:9] = rl.T
                R[9] = rsqh
                R[10] = rsql
                R[11] = 1.0
                R[12] = 1.0
                m[f"rknn_{i}_{b}"] = R
        in_maps.append(m)
    return in_maps


def kernel(**inputs):
    if "nc" not in _NC_CACHE:
        _NC_CACHE["nc"] = _build()
    nc = _NC_CACHE["nc"]
    in_maps = _host_prep(inputs)
    res = run_bass_kernel_spmd(nc, in_maps, list(range(NCORES))).results
    total = 0.0
    for c in range(NCORES):
        total += float(res[c]["loss_part"].sum())
        total += float(res[c]["loss_part3"].sum())
    return np.asarray(total / B, dtype=np.float32)



# revision 30
# speedup vs baseline: 1.1260x; 1.1260x over previous
"""DenoiseNet loss kernel for 8 Trainium2 NeuronCores.

Strategy: pure data parallel over the batch (4 batches/core). PointNet MLP in
fp16 (fp32 PSUM accumulate), exact global BatchNorm via per-layer AllReduce of
(sum, sumsq). KNN argmin via hi/lo-fp16-split matmul of 2q.r-|r|^2; the
matched |v-r*|^2 is extracted by is_equal one-hot + fused multiply-reduce.

v2 scheduling: all KNN matmul groups (argmax pass of iter i, and the
recompute+e pass of iter i-1) are interleaved into the 7 BatchNorm AllReduce
wait windows of iter i's MLP so the PE never idles and stays at max p-state.
DMA triggers ride the (otherwise idle) Sync engine queue; extraction rides
GpSimd; BN affine is applied directly (a*z+c) with 4x-mode vector ops.
"""
import numpy as np

import concourse.bass as bass
import concourse.mybir as mybir
import concourse.tile as tile
from concourse import bacc
from concourse.bass_utils import run_bass_kernel_spmd

dt = mybir.dt
F32 = dt.float32
F16 = dt.float16
AF = mybir.ActivationFunctionType
OP = mybir.AluOpType
AX = mybir.AxisListType

B, N, NCORES = 32, 1000, 8
BL = B // NCORES            # 4 batches per core
PTS = BL * N                # 4000 points per core
NITER = 4
NPTS_GLOBAL = B * N         # 32000 (BN population)
EPS = 1e-5
NOISE_DECAY = 4.0
QT = 125                    # q tile (8 per batch)
RP = 1024                   # padded ref points (24 sentinels), 2 psum banks
RC = 512                    # ref chunk (psum bank: 512 f32)
PTC = 1000                  # pts per copy tile (4 per core)
MMC = 500                   # pts per matmul (psum bank)
NPTC = PTS // PTC           # 4

# (C_in, C_out, has_bn) per layer
LAYERS = [(3, 64, 1), (64, 128, 1), (128, 256, 1), (256, 512, 1),
          (512, 1024, 1), (1024, 512, 1), (512, 256, 1), (256, 3, 0)]
NCI = [max(1, ci // 128) for ci, co, _ in LAYERS]
NCO = [max(1, (co + 127) // 128) for ci, co, _ in LAYERS]
COP = [min(128, co) for ci, co, _ in LAYERS]   # per-chunk out width

RG = [list(range(NCORES))]

_NC_CACHE = {}


def _build(niter=NITER, nlayers=8, do_knn=True, do_ar=True):
    nc = bacc.Bacc(None, target_bir_lowering=False, debug=False)

    x0t_d = nc.dram_tensor("x0t", [3, PTS], F32, kind="ExternalInput")
    sw_d = nc.dram_tensor("sw", [128, 32], F32, kind="ExternalInput")
    sws3_d = nc.dram_tensor("sws3", [3, PTS], F16, kind="ExternalInput")
    db3_d = nc.dram_tensor("db3t", [3, NITER], F32, kind="ExternalInput")
    r_d = [[nc.dram_tensor(f"rknn_{i}_{b}", [11, RP], F16, kind="ExternalInput")
            for b in range(BL)] for i in range(NITER)]
    # whole-layer weights, host-packed as [128, nci, nco, COP]
    w_d = [[nc.dram_tensor(f"w_{i}_{l}", [128, NCI[l], NCO[l], COP[l]], F16,
                           kind="ExternalInput") for l in range(8)]
           for i in range(NITER)]
    gs_d = [[nc.dram_tensor(f"gs_{i}_{l}", [128, 2, NCO[l]], F32,
                            kind="ExternalInput") for l in range(7)]
            for i in range(NITER)]
    loss_d = nc.dram_tensor("loss_part", [128, 1], F32, kind="ExternalOutput")
    loss3_d = nc.dram_tensor("loss_part3", [3, 1], F32, kind="ExternalOutput")
    dbgx_d = nc.dram_tensor("dbg_x", [3, PTS], F32, kind="ExternalOutput")
    dbgs_d = nc.dram_tensor("dbg_S", [128, 32], F32, kind="ExternalOutput")
    dbgz_d = nc.dram_tensor("dbg_z", [128, PTS], F16, kind="ExternalOutput")
    dbgst_d = nc.dram_tensor("dbg_st", [128, 2, 8], F32, kind="ExternalOutput")
    dbgsg_d = nc.dram_tensor("dbg_sg", [128, 2, 8], F32, kind="ExternalOutput")
    dbgm_d = nc.dram_tensor("dbg_m", [128, 32], F32, kind="ExternalOutput")

    with tile.TileContext(nc) as tc:
        with (
            tc.tile_pool(name="sb", bufs=1) as sb,
            tc.tile_pool(name="ps", bufs=4, space="PSUM") as ps,
            tc.tile_pool(name="psk", bufs=1, space="PSUM") as psk,
            tc.tile_pool(name="dram", bufs=2, space="DRAM") as dram,
        ):
            # ---------- persistent setup ----------
            sw_sb = sb.tile([128, 32], F32, tag="sw")
            nc.sync.dma_start(sw_sb[:], sw_d[:])
            db3_sb = sb.tile([3, NITER], F32, tag="db3")
            nc.sync.dma_start(db3_sb[:], db3_d[:])

            x_cur = sb.tile([3, PTS], F32, tag="xA")
            nc.sync.dma_start(x_cur[:], x0t_d[:])

            # Ld (2x hi/lo rows + const -1), parity by iteration; Le for the
            # e-pass of the previous iteration.
            LdP = [sb.tile([11, PTS], F16, tag=f"Ld{p}", name=f"Ld{p}")
                   for p in range(2)]
            Le = sb.tile([11, PTS], F16, tag="Le")
            q16 = sb.tile([3, PTS], F16, tag="q16")
            # constant rows 9:11 (engine writes must start 32-aligned, so
            # build at partition 0 and assemble with DMA)
            nc.gpsimd.memset(q16[0:2, :], -1.0)
            nc.sync.dma_start(LdP[0][9:11, :], q16[0:2, :])
            nc.sync.dma_start(LdP[1][9:11, :], q16[0:2, :])
            nc.gpsimd.memset(q16[0:2, :], 1.0)
            nc.sync.dma_start(Le[9:11, :], q16[0:2, :])

            mP = [sb.tile([128, 32], F32, tag=f"m{p}", name=f"m{p}")
                  for p in range(2)]
            SP = [sb.tile([128, 32], F32, tag=f"S{p}", name=f"S{p}")
                  for p in range(2)]

            eps_sb = sb.tile([128, 1], F32, tag="epsc")
            nc.vector.memset(eps_sb[:], float(EPS))
            licol = sb.tile([128, NITER], F32, tag="licol")
            nc.vector.memset(licol[:], 0.0)
            a3col = sb.tile([3, 2 * NITER], F32, tag="a3col")
            nc.vector.memset(a3col[:], 0.0)

            xf16 = sb.tile([3, PTS], F16, tag="xf")
            nc.scalar.activation(xf16[:], x_cur[:], AF.Copy)

            # per-layer resident weights (one DMA per (iter, layer))
            w_sb = [sb.tile([128, NCI[l], NCO[l], COP[l]], F16, tag=f"wl{l}",
                            name=f"wl{l}") for l in range(8)]
            gs_sb = [sb.tile([128, 2, NCO[l]], F32, tag=f"gsl{l}",
                             name=f"gsl{l}") for l in range(7)]
            r_sb = [[sb.tile([11, RP], F16, tag=f"r{b}_{p}", name=f"r{b}_{p}")
                     for b in range(BL)] for p in range(2)]

            jkV = sb.tile([128, 2000], F16, tag="jkV")
            jkS = sb.tile([128, 2000], F16, tag="jkS")

            # initial loads for iter 0
            for b in range(BL):
                nc.sync.dma_start(r_sb[0][b][:], r_d[0][b][:])
            for l in range(nlayers):
                nc.sync.dma_start(w_sb[l][:], w_d[0][l][:])
                if l < 7:
                    nc.sync.dma_start(gs_sb[l][:], gs_d[0][l][:])

            # build Ld for iter 0
            def build_q(dst, sgn):
                # dst rows: [0:3]=sgn*2x hi, [3:6]=lo, [6:9]=hi, via q16
                nc.scalar.activation(dst[0:3, :], x_cur[:], AF.Copy,
                                     scale=float(sgn) * 2.0)
                nc.vector.scalar_tensor_tensor(q16[:], x_cur[:],
                                               float(sgn) * 2.0, dst[0:3, :],
                                               OP.mult, OP.subtract)
                nc.sync.dma_start(dst[3:6, :], q16[:])
                nc.sync.dma_start(dst[6:9, :], dst[0:3, :])

            build_q(LdP[0], +1.0)

            # ---------- KNN unit emitters ----------
            def knn_unit_C(it, b, qt, tagtog):
                """argmax pass for iter `it` (current x): kpd + row max."""
                p = it % 2
                col = b * 8 + qt
                qsl = slice(b * N + qt * QT, b * N + (qt + 1) * QT)
                kp = psk.tile([QT, 2, RC], F32, tag=("kpd" if tagtog else "kpe"),
                              name=f"kC_{it}_{col}")
                for rt in range(2):
                    nc.tensor.matmul(kp[:, rt, :], LdP[p][0:11, qsl],
                                     r_sb[p][b][0:11, rt * RC:(rt + 1) * RC],
                                     start=True, stop=True)
                nc.vector.tensor_reduce(mP[p][0:QT, col:col + 1], kp[:],
                                        AX.XY, OP.max)

            def knn_unit_AB(it, b, qt, vg):
                """recompute kpd of iter `it` + e-pass, extract matched e."""
                p = it % 2
                col = b * 8 + qt
                qsl = slice(b * N + qt * QT, b * N + (qt + 1) * QT)
                kpd = psk.tile([QT, 2, RC], F32, tag="kpd",
                               name=f"kA_{it}_{col}")
                for rt in range(2):
                    nc.tensor.matmul(kpd[:, rt, :], LdP[p][0:11, qsl],
                                     r_sb[p][b][0:11, rt * RC:(rt + 1) * RC],
                                     start=True, stop=True)
                kpe = psk.tile([QT, 2, RC], F32, tag="kpe",
                               name=f"kB_{it}_{col}")
                for rt in range(2):
                    nc.tensor.matmul(kpe[:, rt, :], Le[0:11, qsl],
                                     r_sb[p][b][0:11, rt * RC:(rt + 1) * RC],
                                     start=True, stop=True)
                e16 = sb.tile([QT, 2, RC], F16, tag="e16", bufs=2,
                              name=f"e16_{it}_{col}")
                nc.scalar.activation(e16[:], kpe[:], AF.Copy)
                nc.vector.scalar_tensor_tensor(
                    e16[:], kpd[:], mP[p][0:QT, col:col + 1], e16[:],
                    OP.is_equal, OP.mult,
                    accum_out=SP[p][0:QT, col:col + 1])

            # window plans: list (per window index 0..6) of unit descriptors
            def window_plan(it):
                units = [(b, qt) for b in range(BL) for qt in range(8)]
                plans = [[] for _ in range(7)]
                if it == 0:
                    counts = [5, 5, 5, 5, 4, 4, 4]
                    k = 0
                    for w, c in enumerate(counts):
                        for _ in range(c):
                            plans[w].append(("C", units[k])); k += 1
                else:
                    ca = [5, 5, 5, 5, 4, 4, 4]
                    cc = [4, 4, 5, 5, 5, 5, 4]
                    ka = kc = 0
                    for w in range(7):
                        for _ in range(ca[w]):
                            plans[w].append(("AB", units[ka])); ka += 1
                        for _ in range(cc[w]):
                            plans[w].append(("C", units[kc])); kc += 1
                return plans

            def emit_window(it, plan):
                tog = True
                vg = False
                for kind, (b, qt) in plan:
                    if not do_knn:
                        continue
                    if kind == "C":
                        knn_unit_C(it, b, qt, tog); tog = not tog
                    else:
                        knn_unit_AB(it - 1, b, qt, vg); vg = not vg

            # ---------- main iteration loop ----------
            for it in range(niter):
                plans = window_plan(it)
                widx = 0
                rhs = [xf16]
                preds = None

                for l in range(nlayers):
                    cin, cout, has_bn = LAYERS[l]
                    nci, nco, cop = NCI[l], NCO[l], COP[l]
                    CIP = min(128, cin)

                    if has_bn:
                        sums = sb.tile([128, nco, 8], F32, tag="sums",
                                       bufs=2)
                        sqacc = sb.tile([128, nco, 2], F32, tag="sqacc",
                                        bufs=2)
                        statsr = sb.tile([128, 2, nco], F32, tag="statsr",
                                         bufs=2)
                        zt = [sb.tile([128, PTS], F16, tag=f"z{l % 2}_{co}",
                                      name=f"z_{it}_{l}_{co}")
                              for co in range(nco)]

                    # prefetch next iteration's copy of this layer's weights
                    # (WAR on w_sb[l] releases after this layer's matmuls)
                    for co in range(nco):
                        CO = min(128, cout - co * 128)
                        for pt in range(8):
                            psl = slice(pt * MMC, (pt + 1) * MMC)
                            zp = ps.tile([128, MMC], F32, tag="zp")
                            for ci in range(nci):
                                nc.tensor.matmul(
                                    zp[0:CO, :],
                                    w_sb[l][0:CIP, ci, co, 0:CO],
                                    rhs[ci][0:CIP, psl],
                                    start=(ci == 0), stop=(ci == nci - 1))
                            if has_bn:
                                nc.scalar.activation(
                                    zt[co][0:CO, psl], zp[0:CO, :],
                                    AF.Copy,
                                    accum_out=sums[0:CO, co, pt:pt + 1])
                                if pt == 3 or pt == 7:
                                    hf = pt // 4
                                    hsl = slice(hf * 2000, (hf + 1) * 2000)
                                    if (co + hf) % 2 == 0:
                                        nc.vector.scalar_tensor_tensor(
                                            jkV[0:CO, :], zt[co][0:CO, hsl],
                                            1.0, zt[co][0:CO, hsl],
                                            OP.mult, OP.mult,
                                            accum_out=sqacc[0:CO, co,
                                                            hf:hf + 1])
                                    else:
                                        nc.scalar.activation(
                                            jkS[0:CO, :], zt[co][0:CO, hsl],
                                            AF.Square,
                                            accum_out=sqacc[0:CO, co,
                                                            hf:hf + 1])
                            else:
                                pbuf = sb.tile([3, MMC], F32, tag="pbuf",
                                               bufs=2)
                                nc.scalar.activation(
                                    pbuf[:, :], zp[0:3, :], AF.Tanh,
                                    bias=db3_sb[:, it:it + 1])
                                # x <- x + pred, in place, per psum tile
                                nc.vector.tensor_tensor(
                                    out=x_cur[:, psl], in0=x_cur[:, psl],
                                    in1=pbuf[:, :], op=OP.add)
                                nc.scalar.activation(xf16[:, psl],
                                                     x_cur[:, psl], AF.Copy)

                    if it + 1 < niter:
                        nc.sync.dma_start(w_sb[l][:], w_d[it + 1][l][:])

                    if not has_bn:
                        break

                    if it == 0 and l == 0:
                        nc.sync.dma_start(dbgz_d[:], zt[0][:])

                    # ---- stats -> AllReduce ----
                    nc.vector.tensor_reduce(statsr[:, 0, :], sums[:], AX.X,
                                            OP.add)
                    nc.vector.tensor_reduce(statsr[:, 1, :], sqacc[:], AX.X,
                                            OP.add)
                    arin = dram.tile([128, 2, nco], F32, tag="arin")
                    arout = dram.tile([128, 2, nco], F32, tag="arout")
                    nc.sync.dma_start(arin[:], statsr[:])
                    if do_ar:
                        nc.gpsimd.collective_compute(
                            "AllReduce", OP.add, replica_groups=RG,
                            ins=[arin.opt()], outs=[arout.opt()])
                    else:
                        nc.sync.dma_start(arout[:], arin[:])
                    statsg = sb.tile([128, 2, nco], F32, tag="statsg", bufs=2)
                    nc.sync.dma_start(statsg[:], arout[:])
                    if it == 0 and l == 0:
                        nc.sync.dma_start(dbgst_d[:, :, 0:nco], statsr[:])
                        nc.sync.dma_start(dbgsg_d[:, :, 0:nco], statsg[:])

                    # ---- fill the AllReduce window with KNN matmuls ----
                    emit_window(it, plans[widx])
                    widx += 1

                    # ---- affine: a = g*rsqrt(var+eps); c = b - mean*a ----
                    gs = gs_sb[l]
                    af = sb.tile([128, 6, nco], F32, tag="af", bufs=2)
                    inv_n = 1.0 / NPTS_GLOBAL
                    nc.vector.tensor_scalar_mul(af[:, 0, :], statsg[:, 0, :],
                                                inv_n)
                    nc.vector.tensor_scalar_mul(af[:, 1, :], statsg[:, 1, :],
                                                inv_n)
                    nc.vector.tensor_tensor(out=af[:, 2, :], in0=af[:, 0, :],
                                            in1=af[:, 0, :], op=OP.mult)
                    nc.vector.tensor_tensor(out=af[:, 1, :], in0=af[:, 1, :],
                                            in1=af[:, 2, :], op=OP.subtract)
                    nc.scalar.activation(af[:, 2, :], af[:, 1, :], AF.Sqrt,
                                         bias=eps_sb[:])
                    nc.vector.reciprocal(af[:, 3, :], af[:, 2, :])
                    nc.vector.tensor_tensor(out=af[:, 4, :], in0=gs[:, 0, :],
                                            in1=af[:, 3, :], op=OP.mult)
                    nc.vector.tensor_tensor(out=af[:, 5, :], in0=af[:, 0, :],
                                            in1=af[:, 4, :], op=OP.mult)
                    nc.vector.tensor_tensor(out=af[:, 5, :], in0=gs[:, 1, :],
                                            in1=af[:, 5, :], op=OP.subtract)

                    # ---- apply affine + relu (first halves, then second) ----
                    for hf in range(2):
                        hsl = slice(hf * 2000, (hf + 1) * 2000)
                        for co in range(nco):
                            CO = min(128, cout - co * 128)
                            nc.vector.tensor_scalar(
                                zt[co][0:CO, hsl], zt[co][0:CO, hsl],
                                af[0:CO, 4, co:co + 1], af[0:CO, 5, co:co + 1],
                                OP.mult, OP.add)
                            nc.vector.tensor_scalar_max(
                                zt[co][0:CO, hsl], zt[co][0:CO, hsl], 0.0)
                    if it + 1 < niter:
                        nc.sync.dma_start(gs_sb[l][:], gs_d[it + 1][l][:])
                    rhs = zt

                if nlayers < 8:
                    continue

                # ---------- iteration boundary ----------
                p = it % 2
                if do_knn:
                    # term2: sum_q sw_q |x_new|^2 via (x*sqrt(sw))^2
                    nc.sync.dma_start(q16[:], sws3_d[:])
                    nc.vector.tensor_tensor(out=q16[:], in0=x_cur[:],
                                            in1=q16[:], op=OP.mult)
                    nc.vector.scalar_tensor_tensor(
                        jkV[0:3, :2000], q16[:, 0:2000], 1.0, q16[:, 0:2000],
                        OP.mult, OP.mult,
                        accum_out=a3col[0:3, 2 * it:2 * it + 1])
                    nc.vector.scalar_tensor_tensor(
                        jkS[0:3, :2000], q16[:, 2000:], 1.0, q16[:, 2000:],
                        OP.mult, OP.mult,
                        accum_out=a3col[0:3, 2 * it + 1:2 * it + 2])
                if it + 1 < niter:
                    # refs for next iteration
                    for b in range(BL):
                        nc.sync.dma_start(r_sb[(it + 1) % 2][b][:],
                                          r_d[it + 1][b][:])
                if do_knn:
                    # Le for THIS iteration's e-pass (runs during next iter /
                    # tail), Ld for next iteration's argmax pass
                    build_q(Le, -1.0)
                    if it + 1 < niter:
                        build_q(LdP[(it + 1) % 2], +1.0)

                if do_knn and it == niter - 1:
                    # tail: extraction units for the last iteration
                    vg = False
                    for b in range(BL):
                        for qt in range(8):
                            knn_unit_AB(it, b, qt, vg); vg = not vg

                if do_knn:
                    # loss contribution of iter `it` is complete once its 32
                    # AB units have run: iters 0..2 complete during iter it+1
                    # windows; emit their licol accum at the NEXT boundary (or
                    # now for the final iteration).
                    done_it = it if it == niter - 1 else (it - 1 if it > 0
                                                         else None)
                    for d in ([done_it] if done_it is not None else []):
                        jk2 = sb.tile([128, 32], F32, tag="jk2", bufs=2)
                        nc.vector.scalar_tensor_tensor(
                            jk2[:], SP[d % 2][:], 1.0, sw_sb[:],
                            OP.mult, OP.mult,
                            accum_out=licol[:, d:d + 1])
                    if it == niter - 1 and niter >= 2:
                        # iter (niter-2)'s units finished during iter niter-1
                        d = niter - 2
                        jk2 = sb.tile([128, 32], F32, tag="jk2", bufs=2)
                        nc.vector.scalar_tensor_tensor(
                            jk2[:], SP[d % 2][:], 1.0, sw_sb[:],
                            OP.mult, OP.mult,
                            accum_out=licol[:, d:d + 1])

            nc.sync.dma_start(dbgx_d[:], x_cur[:])
            if nlayers >= 8 and do_knn:
                nc.sync.dma_start(dbgs_d[:], SP[(niter - 1) % 2][:])
                nc.sync.dma_start(dbgm_d[:], mP[(niter - 1) % 2][:])
            lacc = sb.tile([128, 1], F32, tag="laccA")
            nc.vector.tensor_reduce(lacc[:], licol[:], AX.X, OP.add)
            acc3f = sb.tile([3, 1], F32, tag="acc3A")
            nc.vector.tensor_reduce(acc3f[:], a3col[0:3, :], AX.X, OP.add)
            nc.sync.dma_start(loss_d[:], lacc[:])
            nc.sync.dma_start(loss3_d[:], acc3f[:])
    nc.compile()
    return nc


def _host_prep(inputs):
    """Build per-core input maps."""
    f32 = np.float32
    noisy = np.asarray(inputs["pcl_noisy"], f32)
    clean = np.asarray(inputs["pcl_clean"], f32)
    seeds = np.asarray(inputs["pcl_seeds"], f32)
    std = np.asarray(inputs["pcl_std"], f32)
    noise = np.asarray(inputs["noise"], f32)

    pn = noisy - seeds
    pc = clean - seeds
    sdist = np.sum(pn.astype(np.float64) ** 2, -1, keepdims=True)
    max_sq = sdist[:, -1:, :]
    sw = np.exp(-sdist * 9.0 / max_sq)[..., 0]
    sw = (sw / sw.sum(1, keepdims=True))  # [B, N] float64

    tgts = []
    cur = std.copy()
    for i in range(NITER):
        if i < NITER - 1:
            cur = cur / NOISE_DECAY
            tgts.append(pc + noise[i] * cur[:, None, None])
        else:
            tgts.append(pc.copy())

    shared = {}
    for i in range(NITER):
        for l in range(8):
            key = f'ew{l+1}' if l < 5 else f'dw{l-4}'
            w = np.asarray(inputs[key], f32)[i].astype(np.float16)
            cin, cout = w.shape
            nci, nco, cop = NCI[l], NCO[l], COP[l]
            arr = np.zeros((128, nci, nco, cop), np.float16)
            for ci in range(nci):
                cip = min(128, cin - ci * 128)
                for co in range(nco):
                    cw = min(cop, cout - co * cop)
                    arr[0:cip, ci, co, 0:cw] = w[ci * 128:ci * 128 + cip,
                                                 co * cop:co * cop + cw]
            shared[f"w_{i}_{l}"] = arr
        for l in range(7):
            nco = NCO[l]
            cout = LAYERS[l][1]
            gk = f'eg{l+1}' if l < 5 else f'dg{l-4}'
            hk = f'eh{l+1}' if l < 5 else f'dh{l-4}'
            g = np.asarray(inputs[gk], f32)[i]
            h = np.asarray(inputs[hk], f32)[i]
            arr = np.zeros((128, 2, nco), f32)
            gp = np.zeros(nco * 128, f32); gp[:cout] = g
            hp = np.zeros(nco * 128, f32); hp[:cout] = h
            arr[:, 0, :] = gp.reshape(nco, 128).T
            arr[:, 1, :] = hp.reshape(nco, 128).T
            shared[f"gs_{i}_{l}"] = arr
    shared["db3t"] = np.ascontiguousarray(np.asarray(inputs["db3"], f32).T)

    in_maps = []
    for c in range(NCORES):
        bs = slice(c * BL, (c + 1) * BL)
        m = dict(shared)
        m["x0t"] = np.ascontiguousarray(
            pn[bs].transpose(2, 0, 1).reshape(3, PTS))
        swc = np.zeros((128, 32), f32)
        for b in range(BL):
            for qt in range(8):
                swc[0:QT, b * 8 + qt] = sw[c * BL + b,
                                           qt * QT:(qt + 1) * QT].astype(f32)
        m["sw"] = swc
        m["sws3"] = np.broadcast_to(
            np.sqrt(sw[bs]).reshape(1, PTS), (3, PTS)).astype(np.float16)
        for i in range(NITER):
            for b in range(BL):
                coords = np.concatenate(
                    [tgts[i][c * BL + b].astype(np.float64),
                     np.full((RP - N, 3), 100.0, np.float64)], 0)
                rh = coords.astype(np.float16)
                rl = (coords - rh.astype(np.float64)).astype(np.float16)
                rsq = (coords ** 2).sum(1)
                rsqh = rsq.astype(np.float16)
                rsql = (rsq - rsqh.astype(np.float64)).astype(np.float16)
                R = np.empty((11, RP), np.float16)
                R[0:3] = rh.T
                R[3:6] = rh.T
                R[6:9] = rl.T
                R[9] = rsqh
                R[10] = rsql
                m[f"rknn_{i}_{b}"] = R
        in_maps.append(m)
    return in_maps


def kernel(**inputs):
    if "nc" not in _NC_CACHE:
        _NC_CACHE["nc"] = _build()
    nc = _NC_CACHE["nc"]
    in_maps = _host_prep(inputs)
    res = run_bass_kernel_spmd(nc, in_maps, list(range(NCORES))).results
    total = 0.0
    for c in range(NCORES):
        total += float(res[c]["loss_part"].sum())
        total += float(res[c]["loss_part3"].sum())
    return np.asarray(total / B, dtype=np.float32)


# revision 36
# speedup vs baseline: 1.2194x; 1.0829x over previous
"""DenoiseNet loss kernel for 8 Trainium2 NeuronCores.

Strategy: pure data parallel over the batch (4 batches/core). PointNet MLP in
fp16 (fp32 PSUM accumulate), exact global BatchNorm via per-layer AllReduce of
(sum, sumsq). KNN argmin via hi/lo-fp16-split matmul of 2q.r-|r|^2; the
matched |v-r*|^2 is extracted by is_equal one-hot + fused multiply-reduce.

v2 scheduling: all KNN matmul groups (argmax pass of iter i, and the
recompute+e pass of iter i-1) are interleaved into the 7 BatchNorm AllReduce
wait windows of iter i's MLP so the PE never idles and stays at max p-state.
DMA triggers ride the (otherwise idle) Sync engine queue; extraction rides
GpSimd; BN affine is applied directly (a*z+c) with 4x-mode vector ops.
"""
import numpy as np

import concourse.bass as bass
import concourse.mybir as mybir
import concourse.tile as tile
from concourse import bacc
from concourse.bass_utils import run_bass_kernel_spmd

dt = mybir.dt
F32 = dt.float32
F16 = dt.float16
AF = mybir.ActivationFunctionType
OP = mybir.AluOpType
AX = mybir.AxisListType

B, N, NCORES = 32, 1000, 8
BL = B // NCORES            # 4 batches per core
PTS = BL * N                # 4000 points per core
NITER = 4
NPTS_GLOBAL = B * N         # 32000 (BN population)
EPS = 1e-5
NOISE_DECAY = 4.0
QT = 125                    # q tile (8 per batch)
RP = 1024                   # padded ref points (24 sentinels), 2 psum banks
RC = 512                    # ref chunk (psum bank: 512 f32)
PTC = 1000                  # pts per copy tile (4 per core)
MMC = 500                   # pts per matmul (psum bank)
NPTC = PTS // PTC           # 4

# (C_in, C_out, has_bn) per layer
LAYERS = [(3, 64, 1), (64, 128, 1), (128, 256, 1), (256, 512, 1),
          (512, 1024, 1), (1024, 512, 1), (512, 256, 1), (256, 3, 0)]
NCI = [max(1, ci // 128) for ci, co, _ in LAYERS]
NCO = [max(1, (co + 127) // 128) for ci, co, _ in LAYERS]
COP = [min(128, co) for ci, co, _ in LAYERS]   # per-chunk out width

RG = [list(range(NCORES))]

_NC_CACHE = {}


def _build(niter=NITER, nlayers=8, do_knn=True, do_ar=True):
    nc = bacc.Bacc(None, target_bir_lowering=False, debug=False)

    x0t_d = nc.dram_tensor("x0t", [3, PTS], F32, kind="ExternalInput")
    sw_d = nc.dram_tensor("sw", [128, 32], F32, kind="ExternalInput")
    sws3_d = nc.dram_tensor("sws3", [3, PTS], F16, kind="ExternalInput")
    db3_d = nc.dram_tensor("db3t", [3, NITER], F32, kind="ExternalInput")
    r_d = [[nc.dram_tensor(f"rknn_{i}_{b}", [11, RP], F16, kind="ExternalInput")
            for b in range(BL)] for i in range(NITER)]
    # whole-layer weights, host-packed as [128, nci, nco, COP]
    w_d = [[nc.dram_tensor(f"w_{i}_{l}", [128, NCI[l], NCO[l], COP[l]], F16,
                           kind="ExternalInput") for l in range(8)]
           for i in range(NITER)]
    gs_d = [[nc.dram_tensor(f"gs_{i}_{l}", [128, 2, NCO[l]], F32,
                            kind="ExternalInput") for l in range(7)]
            for i in range(NITER)]
    loss_d = nc.dram_tensor("loss_part", [128, 1], F32, kind="ExternalOutput")
    loss3_d = nc.dram_tensor("loss_part3", [3, 1], F32, kind="ExternalOutput")
    dbgx_d = nc.dram_tensor("dbg_x", [3, PTS], F32, kind="ExternalOutput")
    dbgs_d = nc.dram_tensor("dbg_S", [128, 32], F32, kind="ExternalOutput")
    dbgz_d = nc.dram_tensor("dbg_z", [128, PTS], F16, kind="ExternalOutput")
    dbgst_d = nc.dram_tensor("dbg_st", [128, 2, 8], F32, kind="ExternalOutput")
    dbgsg_d = nc.dram_tensor("dbg_sg", [128, 2, 8], F32, kind="ExternalOutput")
    dbgm_d = nc.dram_tensor("dbg_m", [128, 32], F32, kind="ExternalOutput")

    with tile.TileContext(nc) as tc:
        with (
            tc.tile_pool(name="sb", bufs=1) as sb,
            tc.tile_pool(name="ps", bufs=4, space="PSUM") as ps,
            tc.tile_pool(name="psk", bufs=1, space="PSUM") as psk,
            tc.tile_pool(name="dram", bufs=2, space="DRAM") as dram,
        ):
            # ---------- persistent setup ----------
            sw_sb = sb.tile([128, 32], F32, tag="sw")
            nc.sync.dma_start(sw_sb[:], sw_d[:])
            db3_sb = sb.tile([3, NITER], F32, tag="db3")
            nc.sync.dma_start(db3_sb[:], db3_d[:])

            x_cur = sb.tile([3, PTS], F32, tag="xA")
            nc.sync.dma_start(x_cur[:], x0t_d[:])

            # Ld (2x hi/lo rows + const -1), parity by iteration; Le for the
            # e-pass of the previous iteration.
            LdP = [sb.tile([11, PTS], F16, tag=f"Ld{p}", name=f"Ld{p}")
                   for p in range(2)]
            Le = sb.tile([11, PTS], F16, tag="Le")
            q16 = sb.tile([3, PTS], F16, tag="q16")
            # constant rows 9:11 (engine writes must start 32-aligned, so
            # build at partition 0 and assemble with DMA)
            nc.gpsimd.memset(q16[0:2, :], -1.0)
            nc.sync.dma_start(LdP[0][9:11, :], q16[0:2, :])
            nc.sync.dma_start(LdP[1][9:11, :], q16[0:2, :])
            nc.gpsimd.memset(q16[0:2, :], 1.0)
            nc.sync.dma_start(Le[9:11, :], q16[0:2, :])

            mP = [sb.tile([128, 32], F32, tag=f"m{p}", name=f"m{p}")
                  for p in range(2)]
            SP = [sb.tile([128, 32], F32, tag=f"S{p}", name=f"S{p}")
                  for p in range(2)]

            eps_sb = sb.tile([128, 1], F32, tag="epsc")
            nc.vector.memset(eps_sb[:], float(EPS))
            licol = sb.tile([128, NITER], F32, tag="licol")
            nc.vector.memset(licol[:], 0.0)
            a3col = sb.tile([3, 2 * NITER], F32, tag="a3col")
            nc.vector.memset(a3col[:], 0.0)

            xf16 = sb.tile([3, PTS], F16, tag="xf")
            nc.scalar.activation(xf16[:], x_cur[:], AF.Copy)

            # per-layer resident weights (one DMA per (iter, layer))
            w_sb = [sb.tile([128, NCI[l], NCO[l], COP[l]], F16, tag=f"wl{l}",
                            name=f"wl{l}") for l in range(8)]
            gs_sb = [sb.tile([128, 2, NCO[l]], F32, tag=f"gsl{l}",
                             name=f"gsl{l}") for l in range(7)]
            r_sb = [[sb.tile([11, RP], F16, tag=f"r{b}_{p}", name=f"r{b}_{p}")
                     for b in range(BL)] for p in range(2)]

            jkV = sb.tile([128, 2000], F16, tag="jkV")
            jkS = sb.tile([128, 2000], F16, tag="jkS")

            # initial loads for iter 0
            for b in range(BL):
                nc.sync.dma_start(r_sb[0][b][:], r_d[0][b][:])
            for l in range(nlayers):
                nc.sync.dma_start(w_sb[l][:], w_d[0][l][:])
                if l < 7:
                    nc.sync.dma_start(gs_sb[l][:], gs_d[0][l][:])

            # build Ld for iter 0
            def build_q(dst, sgn):
                # dst rows: [0:3]=sgn*2x hi, [3:6]=lo, [6:9]=hi, via q16
                nc.scalar.activation(dst[0:3, :], x_cur[:], AF.Copy,
                                     scale=float(sgn) * 2.0)
                nc.vector.scalar_tensor_tensor(q16[:], x_cur[:],
                                               float(sgn) * 2.0, dst[0:3, :],
                                               OP.mult, OP.subtract)
                nc.sync.dma_start(dst[3:6, :], q16[:])
                nc.sync.dma_start(dst[6:9, :], dst[0:3, :])

            build_q(LdP[0], +1.0)

            # ---------- KNN unit emitters ----------
            # window_dep: scheduling-only edge that keeps fill units from
            # being hoisted ahead of their AllReduce window
            wdep = [None]

            def _mm(*args, **kw):
                inst = nc.tensor.matmul(*args, **kw)
                if wdep[0] is not None:
                    inst.ins.add_dependency(
                        wdep[0], mybir.DependencyInfo.NO_SYNC_ONLY)
                    wdep[0] = None
                return inst

            def knn_unit_C(it, b, qt, tagtog):
                """argmax pass for iter `it` (current x): kpd + row max."""
                p = it % 2
                col = b * 8 + qt
                qsl = slice(b * N + qt * QT, b * N + (qt + 1) * QT)
                kp = psk.tile([QT, 2, RC], F32, tag=("kpd" if tagtog else "kpe"),
                              name=f"kC_{it}_{col}")
                for rt in range(2):
                    _mm(kp[:, rt, :], LdP[p][0:11, qsl],
                        r_sb[p][b][0:11, rt * RC:(rt + 1) * RC],
                        start=True, stop=True)
                nc.vector.tensor_reduce(mP[p][0:QT, col:col + 1], kp[:],
                                        AX.XY, OP.max)

            def knn_unit_AB(it, b, qt, vg):
                """recompute kpd of iter `it` + e-pass, extract matched e."""
                p = it % 2
                col = b * 8 + qt
                qsl = slice(b * N + qt * QT, b * N + (qt + 1) * QT)
                kpd = psk.tile([QT, 2, RC], F32, tag="kpd",
                               name=f"kA_{it}_{col}")
                for rt in range(2):
                    _mm(kpd[:, rt, :], LdP[p][0:11, qsl],
                        r_sb[p][b][0:11, rt * RC:(rt + 1) * RC],
                        start=True, stop=True)
                kpe = psk.tile([QT, 2, RC], F32, tag="kpe",
                               name=f"kB_{it}_{col}")
                for rt in range(2):
                    _mm(kpe[:, rt, :], Le[0:11, qsl],
                        r_sb[p][b][0:11, rt * RC:(rt + 1) * RC],
                        start=True, stop=True)
                e16 = sb.tile([QT, 2, RC], F16, tag="e16", bufs=2,
                              name=f"e16_{it}_{col}")
                nc.scalar.activation(e16[:], kpe[:], AF.Copy)
                nc.vector.scalar_tensor_tensor(
                    e16[:], kpd[:], mP[p][0:QT, col:col + 1], e16[:],
                    OP.is_equal, OP.mult,
                    accum_out=SP[p][0:QT, col:col + 1])

            # window plans: list (per window index 0..6) of unit descriptors
            def window_plan(it):
                units = [(b, qt) for b in range(BL) for qt in range(8)]
                plans = [[] for _ in range(7)]
                if it == 0:
                    counts = [5, 5, 5, 5, 4, 4, 4]
                    k = 0
                    for w, c in enumerate(counts):
                        for _ in range(c):
                            plans[w].append(("C", units[k])); k += 1
                else:
                    ca = [5, 5, 5, 5, 4, 4, 4]
                    cc = [4, 4, 5, 5, 5, 5, 4]
                    ka = kc = 0
                    for w in range(7):
                        for _ in range(ca[w]):
                            plans[w].append(("AB", units[ka])); ka += 1
                        for _ in range(cc[w]):
                            plans[w].append(("C", units[kc])); kc += 1
                return plans

            def emit_window(it, plan):
                tog = True
                vg = False
                for kind, (b, qt) in plan:
                    if not do_knn:
                        continue
                    if kind == "C":
                        knn_unit_C(it, b, qt, tog); tog = not tog
                    else:
                        knn_unit_AB(it - 1, b, qt, vg); vg = not vg

            # ---------- main iteration loop ----------
            for it in range(niter):
                plans = window_plan(it)
                widx = 0
                rhs = [xf16]
                preds = None

                for l in range(nlayers):
                    cin, cout, has_bn = LAYERS[l]
                    nci, nco, cop = NCI[l], NCO[l], COP[l]
                    CIP = min(128, cin)

                    if has_bn:
                        sums = sb.tile([128, nco, 8], F32, tag="sums",
                                       bufs=2)
                        sqacc = sb.tile([128, nco, 2], F32, tag="sqacc",
                                        bufs=2)
                        statsr = sb.tile([128, 2, nco], F32, tag="statsr",
                                         bufs=2)
                        zt = [sb.tile([128, PTS], F16, tag=f"z{l % 2}_{co}",
                                      name=f"z_{it}_{l}_{co}")
                              for co in range(nco)]

                    # prefetch next iteration's copy of this layer's weights
                    # (WAR on w_sb[l] releases after this layer's matmuls)
                    for co in range(nco):
                        CO = min(128, cout - co * 128)
                        for pt in range(8):
                            psl = slice(pt * MMC, (pt + 1) * MMC)
                            zp = ps.tile([128, MMC], F32, tag="zp")
                            for ci in range(nci):
                                nc.tensor.matmul(
                                    zp[0:CO, :],
                                    w_sb[l][0:CIP, ci, co, 0:CO],
                                    rhs[ci][0:CIP, psl],
                                    start=(ci == 0), stop=(ci == nci - 1))
                            if has_bn:
                                last_cp = nc.scalar.activation(
                                    zt[co][0:CO, psl], zp[0:CO, :],
                                    AF.Copy,
                                    accum_out=sums[0:CO, co, pt:pt + 1])
                                if pt == 3 or pt == 7:
                                    hf = pt // 4
                                    hsl = slice(hf * 2000, (hf + 1) * 2000)
                                    if (co + hf) % 2 == 0:
                                        nc.vector.scalar_tensor_tensor(
                                            jkV[0:CO, :], zt[co][0:CO, hsl],
                                            1.0, zt[co][0:CO, hsl],
                                            OP.mult, OP.mult,
                                            accum_out=sqacc[0:CO, co,
                                                            hf:hf + 1])
                                    else:
                                        nc.scalar.activation(
                                            jkS[0:CO, :], zt[co][0:CO, hsl],
                                            AF.Square,
                                            accum_out=sqacc[0:CO, co,
                                                            hf:hf + 1])
                            else:
                                pbuf = sb.tile([3, MMC], F32, tag="pbuf",
                                               bufs=2)
                                nc.scalar.activation(
                                    pbuf[:, :], zp[0:3, :], AF.Tanh,
                                    bias=db3_sb[:, it:it + 1])
                                # x <- x + pred, in place, per psum tile
                                nc.vector.tensor_tensor(
                                    out=x_cur[:, psl], in0=x_cur[:, psl],
                                    in1=pbuf[:, :], op=OP.add)
                                nc.scalar.activation(xf16[:, psl],
                                                     x_cur[:, psl], AF.Copy)

                    if it + 1 < niter:
                        nc.sync.dma_start(w_sb[l][:], w_d[it + 1][l][:])

                    if not has_bn:
                        break

                    if it == 0 and l == 0:
                        nc.sync.dma_start(dbgz_d[:], zt[0][:])

                    # ---- stats -> AllReduce ----
                    nc.vector.tensor_reduce(statsr[:, 0, :], sums[:], AX.X,
                                            OP.add)
                    nc.vector.tensor_reduce(statsr[:, 1, :], sqacc[:], AX.X,
                                            OP.add)
                    arin = dram.tile([128, 2, nco], F32, tag="arin")
                    arout = dram.tile([128, 2, nco], F32, tag="arout")
                    nc.sync.dma_start(arin[:], statsr[:])
                    if do_ar:
                        nc.gpsimd.collective_compute(
                            "AllReduce", OP.add, replica_groups=RG,
                            ins=[arin.opt()], outs=[arout.opt()])
                    else:
                        nc.sync.dma_start(arout[:], arin[:])
                    statsg = sb.tile([128, 2, nco], F32, tag="statsg", bufs=2)
                    nc.sync.dma_start(statsg[:], arout[:])
                    if it == 0 and l == 0:
                        nc.sync.dma_start(dbgst_d[:, :, 0:nco], statsr[:])
                        nc.sync.dma_start(dbgsg_d[:, :, 0:nco], statsg[:])

                    # ---- fill the AllReduce window with KNN matmuls ----
                    wdep[0] = last_cp.ins.name
                    emit_window(it, plans[widx])
                    wdep[0] = None
                    widx += 1

                    # ---- affine: a = g*rsqrt(var+eps); c = b - mean*a ----
                    gs = gs_sb[l]
                    af = sb.tile([128, 6, nco], F32, tag="af", bufs=2)
                    inv_n = 1.0 / NPTS_GLOBAL
                    nc.vector.tensor_scalar_mul(af[:, 0, :], statsg[:, 0, :],
                                                inv_n)
                    nc.vector.tensor_scalar_mul(af[:, 1, :], statsg[:, 1, :],
                                                inv_n)
                    nc.vector.tensor_tensor(out=af[:, 2, :], in0=af[:, 0, :],
                                            in1=af[:, 0, :], op=OP.mult)
                    nc.vector.tensor_tensor(out=af[:, 1, :], in0=af[:, 1, :],
                                            in1=af[:, 2, :], op=OP.subtract)
                    nc.scalar.activation(af[:, 2, :], af[:, 1, :], AF.Sqrt,
                                         bias=eps_sb[:])
                    nc.vector.reciprocal(af[:, 3, :], af[:, 2, :])
                    nc.vector.tensor_tensor(out=af[:, 4, :], in0=gs[:, 0, :],
                                            in1=af[:, 3, :], op=OP.mult)
                    nc.vector.tensor_tensor(out=af[:, 5, :], in0=af[:, 0, :],
                                            in1=af[:, 4, :], op=OP.mult)
                    nc.vector.tensor_tensor(out=af[:, 5, :], in0=gs[:, 1, :],
                                            in1=af[:, 5, :], op=OP.subtract)

                    # ---- apply affine + relu (first halves, then second) ----
                    for hf in range(2):
                        hsl = slice(hf * 2000, (hf + 1) * 2000)
                        for co in range(nco):
                            CO = min(128, cout - co * 128)
                            nc.vector.tensor_scalar(
                                zt[co][0:CO, hsl], zt[co][0:CO, hsl],
                                af[0:CO, 4, co:co + 1], af[0:CO, 5, co:co + 1],
                                OP.mult, OP.add)
                            nc.vector.tensor_scalar_max(
                                zt[co][0:CO, hsl], zt[co][0:CO, hsl], 0.0)
                    if it + 1 < niter:
                        nc.sync.dma_start(gs_sb[l][:], gs_d[it + 1][l][:])
                    rhs = zt

                if nlayers < 8:
                    continue

                # ---------- iteration boundary ----------
                p = it % 2
                if do_knn:
                    # term2: sum_q sw_q |x_new|^2 via (x*sqrt(sw))^2
                    nc.sync.dma_start(q16[:], sws3_d[:])
                    nc.vector.tensor_tensor(out=q16[:], in0=x_cur[:],
                                            in1=q16[:], op=OP.mult)
                    nc.vector.scalar_tensor_tensor(
                        jkV[0:3, :2000], q16[:, 0:2000], 1.0, q16[:, 0:2000],
                        OP.mult, OP.mult,
                        accum_out=a3col[0:3, 2 * it:2 * it + 1])
                    nc.vector.scalar_tensor_tensor(
                        jkS[0:3, :2000], q16[:, 2000:], 1.0, q16[:, 2000:],
                        OP.mult, OP.mult,
                        accum_out=a3col[0:3, 2 * it + 1:2 * it + 2])
                if it + 1 < niter:
                    # refs for next iteration
                    for b in range(BL):
                        nc.sync.dma_start(r_sb[(it + 1) % 2][b][:],
                                          r_d[it + 1][b][:])
                if do_knn:
                    # Le for THIS iteration's e-pass (runs during next iter /
                    # tail), Ld for next iteration's argmax pass
                    build_q(Le, -1.0)
                    if it + 1 < niter:
                        build_q(LdP[(it + 1) % 2], +1.0)

                if do_knn and it == niter - 1:
                    # tail: extraction units for the last iteration
                    vg = False
                    for b in range(BL):
                        for qt in range(8):
                            knn_unit_AB(it, b, qt, vg); vg = not vg

                if do_knn:
                    # loss contribution of iter `it` is complete once its 32
                    # AB units have run: iters 0..2 complete during iter it+1
                    # windows; emit their licol accum at the NEXT boundary (or
                    # now for the final iteration).
                    done_it = it if it == niter - 1 else (it - 1 if it > 0
                                                         else None)
                    for d in ([done_it] if done_it is not None else []):
                        jk2 = sb.tile([128, 32], F32, tag="jk2", bufs=2)
                        nc.vector.scalar_tensor_tensor(
                            jk2[:], SP[d % 2][:], 1.0, sw_sb[:],
                            OP.mult, OP.mult,
                            accum_out=licol[:, d:d + 1])
                    if it == niter - 1 and niter >= 2:
                        # iter (niter-2)'s units finished during iter niter-1
                        d = niter - 2
                        jk2 = sb.tile([128, 32], F32, tag="jk2", bufs=2)
                        nc.vector.scalar_tensor_tensor(
                            jk2[:], SP[d % 2][:], 1.0, sw_sb[:],
                            OP.mult, OP.mult,
                            accum_out=licol[:, d:d + 1])

            nc.sync.dma_start(dbgx_d[:], x_cur[:])
            if nlayers >= 8 and do_knn:
                nc.sync.dma_start(dbgs_d[:], SP[(niter - 1) % 2][:])
                nc.sync.dma_start(dbgm_d[:], mP[(niter - 1) % 2][:])
            lacc = sb.tile([128, 1], F32, tag="laccA")
            nc.vector.tensor_reduce(lacc[:], licol[:], AX.X, OP.add)
            acc3f = sb.tile([3, 1], F32, tag="acc3A")
            nc.vector.tensor_reduce(acc3f[:], a3col[0:3, :], AX.X, OP.add)
            nc.sync.dma_start(loss_d[:], lacc[:])
            nc.sync.dma_start(loss3_d[:], acc3f[:])
    nc.compile()
    return nc


def _host_prep(inputs):
    """Build per-core input maps."""
    f32 = np.float32
    noisy = np.asarray(inputs["pcl_noisy"], f32)
    clean = np.asarray(inputs["pcl_clean"], f32)
    seeds = np.asarray(inputs["pcl_seeds"], f32)
    std = np.asarray(inputs["pcl_std"], f32)
    noise = np.asarray(inputs["noise"], f32)

    pn = noisy - seeds
    pc = clean - seeds
    sdist = np.sum(pn.astype(np.float64) ** 2, -1, keepdims=True)
    max_sq = sdist[:, -1:, :]
    sw = np.exp(-sdist * 9.0 / max_sq)[..., 0]
    sw = (sw / sw.sum(1, keepdims=True))  # [B, N] float64

    tgts = []
    cur = std.copy()
    for i in range(NITER):
        if i < NITER - 1:
            cur = cur / NOISE_DECAY
            tgts.append(pc + noise[i] * cur[:, None, None])
        else:
            tgts.append(pc.copy())

    shared = {}
    for i in range(NITER):
        for l in range(8):
            key = f'ew{l+1}' if l < 5 else f'dw{l-4}'
            w = np.asarray(inputs[key], f32)[i].astype(np.float16)
            cin, cout = w.shape
            nci, nco, cop = NCI[l], NCO[l], COP[l]
            arr = np.zeros((128, nci, nco, cop), np.float16)
            for ci in range(nci):
                cip = min(128, cin - ci * 128)
                for co in range(nco):
                    cw = min(cop, cout - co * cop)
                    arr[0:cip, ci, co, 0:cw] = w[ci * 128:ci * 128 + cip,
                                                 co * cop:co * cop + cw]
            shared[f"w_{i}_{l}"] = arr
        for l in range(7):
            nco = NCO[l]
            cout = LAYERS[l][1]
            gk = f'eg{l+1}' if l < 5 else f'dg{l-4}'
            hk = f'eh{l+1}' if l < 5 else f'dh{l-4}'
            g = np.asarray(inputs[gk], f32)[i]
            h = np.asarray(inputs[hk], f32)[i]
            arr = np.zeros((128, 2, nco), f32)
            gp = np.zeros(nco * 128, f32); gp[:cout] = g
            hp = np.zeros(nco * 128, f32); hp[:cout] = h
            arr[:, 0, :] = gp.reshape(nco, 128).T
            arr[:, 1, :] = hp.reshape(nco, 128).T
            shared[f"gs_{i}_{l}"] = arr
    shared["db3t"] = np.ascontiguousarray(np.asarray(inputs["db3"], f32).T)

    in_maps = []
    for c in range(NCORES):
        bs = slice(c * BL, (c + 1) * BL)
        m = dict(shared)
        m["x0t"] = np.ascontiguousarray(
            pn[bs].transpose(2, 0, 1).reshape(3, PTS))
        swc = np.zeros((128, 32), f32)
        for b in range(BL):
            for qt in range(8):
                swc[0:QT, b * 8 + qt] = sw[c * BL + b,
                                           qt * QT:(qt + 1) * QT].astype(f32)
        m["sw"] = swc
        m["sws3"] = np.broadcast_to(
            np.sqrt(sw[bs]).reshape(1, PTS), (3, PTS)).astype(np.float16)
        for i in range(NITER):
            for b in range(BL):
                coords = np.concatenate(
                    [tgts[i][c * BL + b].astype(np.float64),
                     np.full((RP - N, 3), 100.0, np.float64)], 0)
                rh = coords.astype(np.float16)
                rl = (coords - rh.astype(np.float64)).astype(np.float16)
                rsq = (coords ** 2).sum(1)
                rsqh = rsq.astype(np.float16)
                rsql = (rsq - rsqh.astype(np.float64)).astype(np.float16)
                R = np.empty((11, RP), np.float16)
                R[0:3] = rh.T
                R[3:6] = rh.T
                R[6:9] = rl.T
                R[9] = rsqh
                R[10] = rsql
                m[f"rknn_{i}_{b}"] = R
        in_maps.append(m)
    return in_maps


def kernel(**inputs):
    if "nc" not in _NC_CACHE:
        _NC_CACHE["nc"] = _build()
    nc = _NC_CACHE["nc"]
    in_maps = _host_prep(inputs)
    res = run_bass_kernel_spmd(nc, in_maps, list(range(NCORES))).results
    total = 0.0
    for c in range(NCORES):
        total += float(res[c]["loss_part"].sum())
        total += float(res[c]["loss_part3"].sum())
    return np.asarray(total / B, dtype=np.float32)


# revision 39
# speedup vs baseline: 1.2217x; 1.0019x over previous
"""DenoiseNet loss kernel for 8 Trainium2 NeuronCores.

Strategy: pure data parallel over the batch (4 batches/core). PointNet MLP in
fp16 (fp32 PSUM accumulate), exact global BatchNorm via per-layer AllReduce of
(sum, sumsq). KNN argmin via hi/lo-fp16-split matmul of 2q.r-|r|^2; the
matched |v-r*|^2 is extracted by is_equal one-hot + fused multiply-reduce.

v2 scheduling: all KNN matmul groups (argmax pass of iter i, and the
recompute+e pass of iter i-1) are interleaved into the 7 BatchNorm AllReduce
wait windows of iter i's MLP so the PE never idles and stays at max p-state.
DMA triggers ride the (otherwise idle) Sync engine queue; extraction rides
GpSimd; BN affine is applied directly (a*z+c) with 4x-mode vector ops.
"""
import numpy as np

import concourse.bass as bass
import concourse.mybir as mybir
import concourse.tile as tile
from concourse import bacc
from concourse.bass_utils import run_bass_kernel_spmd

dt = mybir.dt
F32 = dt.float32
F16 = dt.float16
AF = mybir.ActivationFunctionType
OP = mybir.AluOpType
AX = mybir.AxisListType

B, N, NCORES = 32, 1000, 8
BL = B // NCORES            # 4 batches per core
PTS = BL * N                # 4000 points per core
NITER = 4
NPTS_GLOBAL = B * N         # 32000 (BN population)
EPS = 1e-5
NOISE_DECAY = 4.0
QT = 125                    # q tile (8 per batch)
RP = 1024                   # padded ref points (24 sentinels), 2 psum banks
RC = 512                    # ref chunk (psum bank: 512 f32)
PTC = 1000                  # pts per copy tile (4 per core)
MMC = 500                   # pts per matmul (psum bank)
NPTC = PTS // PTC           # 4

# (C_in, C_out, has_bn) per layer
LAYERS = [(3, 64, 1), (64, 128, 1), (128, 256, 1), (256, 512, 1),
          (512, 1024, 1), (1024, 512, 1), (512, 256, 1), (256, 3, 0)]
NCI = [max(1, ci // 128) for ci, co, _ in LAYERS]
NCO = [max(1, (co + 127) // 128) for ci, co, _ in LAYERS]
COP = [min(128, co) for ci, co, _ in LAYERS]   # per-chunk out width

RG = [list(range(NCORES))]

_NC_CACHE = {}


def _build(niter=NITER, nlayers=8, do_knn=True, do_ar=True):
    nc = bacc.Bacc(None, target_bir_lowering=False, debug=False)

    x0t_d = nc.dram_tensor("x0t", [3, PTS], F32, kind="ExternalInput")
    sw_d = nc.dram_tensor("sw", [128, 32], F32, kind="ExternalInput")
    sws3_d = nc.dram_tensor("sws3", [3, PTS], F16, kind="ExternalInput")
    db3_d = nc.dram_tensor("db3t", [3, NITER], F32, kind="ExternalInput")
    r_d = [[nc.dram_tensor(f"rknn_{i}_{b}", [11, RP], F16, kind="ExternalInput")
            for b in range(BL)] for i in range(NITER)]
    # whole-layer weights, host-packed as [128, nci, nco, COP]
    w_d = [[nc.dram_tensor(f"w_{i}_{l}", [128, NCI[l], NCO[l], COP[l]], F16,
                           kind="ExternalInput") for l in range(8)]
           for i in range(NITER)]
    gs_d = [[nc.dram_tensor(f"gs_{i}_{l}", [128, 2, NCO[l]], F32,
                            kind="ExternalInput") for l in range(7)]
            for i in range(NITER)]
    loss_d = nc.dram_tensor("loss_part", [128, 1], F32, kind="ExternalOutput")
    loss3_d = nc.dram_tensor("loss_part3", [3, 1], F32, kind="ExternalOutput")
    dbgx_d = nc.dram_tensor("dbg_x", [3, PTS], F32, kind="ExternalOutput")
    dbgs_d = nc.dram_tensor("dbg_S", [128, 32], F32, kind="ExternalOutput")
    dbgz_d = nc.dram_tensor("dbg_z", [128, PTS], F16, kind="ExternalOutput")
    dbgst_d = nc.dram_tensor("dbg_st", [128, 2, 8], F32, kind="ExternalOutput")
    dbgsg_d = nc.dram_tensor("dbg_sg", [128, 2, 8], F32, kind="ExternalOutput")
    dbgm_d = nc.dram_tensor("dbg_m", [128, 32], F32, kind="ExternalOutput")

    with tile.TileContext(nc) as tc:
        with (
            tc.tile_pool(name="sb", bufs=1) as sb,
            tc.tile_pool(name="ps", bufs=2, space="PSUM") as ps,
            tc.tile_pool(name="psk", bufs=1, space="PSUM") as psk,
            tc.tile_pool(name="dram", bufs=2, space="DRAM") as dram,
        ):
            # ---------- persistent setup ----------
            sw_sb = sb.tile([128, 32], F32, tag="sw")
            nc.sync.dma_start(sw_sb[:], sw_d[:])
            db3_sb = sb.tile([3, NITER], F32, tag="db3")
            nc.sync.dma_start(db3_sb[:], db3_d[:])

            x_cur = sb.tile([3, PTS], F32, tag="xA")
            nc.sync.dma_start(x_cur[:], x0t_d[:])

            # Ld (2x hi/lo rows + const -1), parity by iteration; Le for the
            # e-pass of the previous iteration.
            LdP = [sb.tile([11, PTS], F16, tag=f"Ld{p}", name=f"Ld{p}")
                   for p in range(2)]
            Le = sb.tile([11, PTS], F16, tag="Le")
            q16 = sb.tile([3, PTS], F16, tag="q16")
            # constant rows 9:11 (engine writes must start 32-aligned, so
            # build at partition 0 and assemble with DMA)
            nc.gpsimd.memset(q16[0:2, :], -1.0)
            nc.sync.dma_start(LdP[0][9:11, :], q16[0:2, :])
            nc.sync.dma_start(LdP[1][9:11, :], q16[0:2, :])
            nc.gpsimd.memset(q16[0:2, :], 1.0)
            nc.sync.dma_start(Le[9:11, :], q16[0:2, :])

            mP = [sb.tile([128, 32], F32, tag=f"m{p}", name=f"m{p}")
                  for p in range(2)]
            SP = [sb.tile([128, 32], F32, tag=f"S{p}", name=f"S{p}")
                  for p in range(2)]

            eps_sb = sb.tile([128, 1], F32, tag="epsc")
            nc.vector.memset(eps_sb[:], float(EPS))
            licol = sb.tile([128, NITER], F32, tag="licol")
            nc.vector.memset(licol[:], 0.0)
            a3col = sb.tile([3, 2 * NITER], F32, tag="a3col")
            nc.vector.memset(a3col[:], 0.0)

            xf16 = sb.tile([3, PTS], F16, tag="xf")
            nc.scalar.activation(xf16[:], x_cur[:], AF.Copy)

            # per-layer resident weights (one DMA per (iter, layer))
            w_sb = [sb.tile([128, NCI[l], NCO[l], COP[l]], F16, tag=f"wl{l}",
                            name=f"wl{l}") for l in range(8)]
            gs_sb = [sb.tile([128, 2, NCO[l]], F32, tag=f"gsl{l}",
                             name=f"gsl{l}") for l in range(7)]
            r_sb = [[sb.tile([11, RP], F16, tag=f"r{b}_{p}", name=f"r{b}_{p}")
                     for b in range(BL)] for p in range(2)]

            jkV = sb.tile([128, 2000], F16, tag="jkV")
            jkS = sb.tile([128, 2000], F16, tag="jkS")

            # initial loads for iter 0
            for b in range(BL):
                nc.sync.dma_start(r_sb[0][b][:], r_d[0][b][:])
            for l in range(nlayers):
                nc.sync.dma_start(w_sb[l][:], w_d[0][l][:])
                if l < 7:
                    nc.sync.dma_start(gs_sb[l][:], gs_d[0][l][:])

            # build Ld for iter 0
            def build_q(dst, sgn):
                # dst rows: [0:3]=sgn*2x hi, [3:6]=lo, [6:9]=hi, via q16
                nc.scalar.activation(dst[0:3, :], x_cur[:], AF.Copy,
                                     scale=float(sgn) * 2.0)
                nc.vector.scalar_tensor_tensor(q16[:], x_cur[:],
                                               float(sgn) * 2.0, dst[0:3, :],
                                               OP.mult, OP.subtract)
                nc.sync.dma_start(dst[3:6, :], q16[:])
                nc.sync.dma_start(dst[6:9, :], dst[0:3, :])

            build_q(LdP[0], +1.0)

            # ---------- KNN unit emitters ----------
            # window_dep: scheduling-only edge that keeps fill units from
            # being hoisted ahead of their AllReduce window
            wdep = [None]

            def _mm(*args, **kw):
                inst = nc.tensor.matmul(*args, **kw)
                if wdep[0] is not None:
                    inst.ins.add_dependency(
                        wdep[0], mybir.DependencyInfo.NO_SYNC_ONLY)
                    wdep[0] = None
                return inst

            def knn_unit_C(it, b, qt, tagtog):
                """argmax pass for iter `it` (current x): kpd + row max."""
                p = it % 2
                col = b * 8 + qt
                qsl = slice(b * N + qt * QT, b * N + (qt + 1) * QT)
                kp = psk.tile([QT, 2, RC], F32, tag=("kpd" if tagtog else "kpe"),
                              name=f"kC_{it}_{col}")
                for rt in range(2):
                    _mm(kp[:, rt, :], LdP[p][0:11, qsl],
                        r_sb[p][b][0:11, rt * RC:(rt + 1) * RC],
                        start=True, stop=True)
                nc.vector.tensor_reduce(mP[p][0:QT, col:col + 1], kp[:],
                                        AX.XY, OP.max)

            def knn_unit_AB(it, b, qt, vg):
                """recompute kpd of iter `it` + e-pass, extract matched e."""
                p = it % 2
                col = b * 8 + qt
                qsl = slice(b * N + qt * QT, b * N + (qt + 1) * QT)
                kpd = psk.tile([QT, 2, RC], F32, tag="kpd",
                               name=f"kA_{it}_{col}")
                for rt in range(2):
                    _mm(kpd[:, rt, :], LdP[p][0:11, qsl],
                        r_sb[p][b][0:11, rt * RC:(rt + 1) * RC],
                        start=True, stop=True)
                kpe = psk.tile([QT, 2, RC], F32, tag="kpe",
                               name=f"kB_{it}_{col}")
                for rt in range(2):
                    _mm(kpe[:, rt, :], Le[0:11, qsl],
                        r_sb[p][b][0:11, rt * RC:(rt + 1) * RC],
                        start=True, stop=True)
                e16 = sb.tile([QT, 2, RC], F16, tag="e16", bufs=2,
                              name=f"e16_{it}_{col}")
                nc.scalar.activation(e16[:], kpe[:], AF.Copy)
                nc.vector.scalar_tensor_tensor(
                    e16[:], kpd[:], mP[p][0:QT, col:col + 1], e16[:],
                    OP.is_equal, OP.mult,
                    accum_out=SP[p][0:QT, col:col + 1])

            # window plans: list (per window index 0..6) of unit descriptors
            def window_plan(it):
                units = [(b, qt) for b in range(BL) for qt in range(8)]
                plans = [[] for _ in range(7)]
                if it == 0:
                    counts = [5, 5, 5, 5, 4, 4, 4]
                    k = 0
                    for w, c in enumerate(counts):
                        for _ in range(c):
                            plans[w].append(("C", units[k])); k += 1
                else:
                    ca = [5, 5, 5, 5, 4, 4, 4]
                    cc = [4, 4, 5, 5, 5, 5, 4]
                    ka = kc = 0
                    for w in range(7):
                        for _ in range(ca[w]):
                            plans[w].append(("AB", units[ka])); ka += 1
                        for _ in range(cc[w]):
                            plans[w].append(("C", units[kc])); kc += 1
                return plans

            def emit_window(it, plan):
                tog = True
                vg = False
                for kind, (b, qt) in plan:
                    if not do_knn:
                        continue
                    if kind == "C":
                        knn_unit_C(it, b, qt, tog); tog = not tog
                    else:
                        knn_unit_AB(it - 1, b, qt, vg); vg = not vg

            # ---------- main iteration loop ----------
            for it in range(niter):
                plans = window_plan(it)
                widx = 0
                rhs = [xf16]
                preds = None

                for l in range(nlayers):
                    cin, cout, has_bn = LAYERS[l]
                    nci, nco, cop = NCI[l], NCO[l], COP[l]
                    CIP = min(128, cin)

                    if has_bn:
                        sums = sb.tile([128, nco, NPTC], F32, tag="sums",
                                       bufs=2)
                        sqacc = sb.tile([128, nco, 2], F32, tag="sqacc",
                                        bufs=2)
                        statsr = sb.tile([128, 2, nco], F32, tag="statsr",
                                         bufs=2)
                        zt = [sb.tile([128, PTS], F16, tag=f"z{l % 2}_{co}",
                                      name=f"z_{it}_{l}_{co}")
                              for co in range(nco)]

                    # prefetch next iteration's copy of this layer's weights
                    # (WAR on w_sb[l] releases after this layer's matmuls)
                    for co in range(nco):
                        CO = min(128, cout - co * 128)
                        for ptc in range(NPTC):
                            # two bank-aligned accumulation groups per tile;
                            # matmuls write 500-col slices at 512-col strides
                            zp = ps.tile([128, 2, RC], F32, tag="zp")
                            for h in range(2):
                                psl = slice(ptc * PTC + h * MMC,
                                            ptc * PTC + (h + 1) * MMC)
                                for ci in range(nci):
                                    nc.tensor.matmul(
                                        zp[0:CO, h, 0:MMC],
                                        w_sb[l][0:CIP, ci, co, 0:CO],
                                        rhs[ci][0:CIP, psl],
                                        start=(ci == 0), stop=(ci == nci - 1))
                            ptsl = slice(ptc * PTC, (ptc + 1) * PTC)
                            if has_bn:
                                last_cp = nc.scalar.activation(
                                    zt[co][0:CO, ptsl], zp[0:CO, :, 0:MMC],
                                    AF.Copy,
                                    accum_out=sums[0:CO, co, ptc:ptc + 1])
                                if ptc == 1 or ptc == 3:
                                    hf = ptc // 2
                                    hsl = slice(hf * 2000, (hf + 1) * 2000)
                                    if (co + hf) % 2 == 0:
                                        nc.vector.scalar_tensor_tensor(
                                            jkV[0:CO, :], zt[co][0:CO, hsl],
                                            1.0, zt[co][0:CO, hsl],
                                            OP.mult, OP.mult,
                                            accum_out=sqacc[0:CO, co,
                                                            hf:hf + 1])
                                    else:
                                        nc.scalar.activation(
                                            jkS[0:CO, :], zt[co][0:CO, hsl],
                                            AF.Square,
                                            accum_out=sqacc[0:CO, co,
                                                            hf:hf + 1])
                            else:
                                pbuf = sb.tile([3, PTC], F32, tag="pbuf",
                                               bufs=2)
                                nc.scalar.activation(
                                    pbuf[:, :], zp[0:3, :, 0:MMC], AF.Tanh,
                                    bias=db3_sb[:, it:it + 1])
                                # x <- x + pred, in place, per copy tile
                                nc.vector.tensor_tensor(
                                    out=x_cur[:, ptsl], in0=x_cur[:, ptsl],
                                    in1=pbuf[:, :], op=OP.add)
                                nc.scalar.activation(xf16[:, ptsl],
                                                     x_cur[:, ptsl], AF.Copy)

                    if it + 1 < niter:
                        nc.sync.dma_start(w_sb[l][:], w_d[it + 1][l][:])

                    if not has_bn:
                        break

                    if it == 0 and l == 0:
                        nc.sync.dma_start(dbgz_d[:], zt[0][:])

                    # ---- stats -> AllReduce ----
                    nc.vector.tensor_reduce(statsr[:, 0, :], sums[:], AX.X,
                                            OP.add)
                    nc.vector.tensor_reduce(statsr[:, 1, :], sqacc[:], AX.X,
                                            OP.add)
                    arin = dram.tile([128, 2, nco], F32, tag="arin")
                    arout = dram.tile([128, 2, nco], F32, tag="arout")
                    nc.sync.dma_start(arin[:], statsr[:])
                    if do_ar:
                        nc.gpsimd.collective_compute(
                            "AllReduce", OP.add, replica_groups=RG,
                            ins=[arin.opt()], outs=[arout.opt()])
                    else:
                        nc.sync.dma_start(arout[:], arin[:])
                    statsg = sb.tile([128, 2, nco], F32, tag="statsg", bufs=2)
                    nc.sync.dma_start(statsg[:], arout[:])
                    if it == 0 and l == 0:
                        nc.sync.dma_start(dbgst_d[:, :, 0:nco], statsr[:])
                        nc.sync.dma_start(dbgsg_d[:, :, 0:nco], statsg[:])

                    # ---- fill the AllReduce window with KNN matmuls ----
                    wdep[0] = last_cp.ins.name
                    emit_window(it, plans[widx])
                    wdep[0] = None
                    widx += 1

                    # ---- affine: a = g*rsqrt(var+eps); c = b - mean*a ----
                    gs = gs_sb[l]
                    af = sb.tile([128, 6, nco], F32, tag="af", bufs=2)
                    inv_n = 1.0 / NPTS_GLOBAL
                    nc.vector.tensor_scalar_mul(af[:, 0, :], statsg[:, 0, :],
                                                inv_n)
                    nc.vector.tensor_scalar_mul(af[:, 1, :], statsg[:, 1, :],
                                                inv_n)
                    nc.vector.tensor_tensor(out=af[:, 2, :], in0=af[:, 0, :],
                                            in1=af[:, 0, :], op=OP.mult)
                    nc.vector.tensor_tensor(out=af[:, 1, :], in0=af[:, 1, :],
                                            in1=af[:, 2, :], op=OP.subtract)
                    nc.scalar.activation(af[:, 2, :], af[:, 1, :], AF.Sqrt,
                                         bias=eps_sb[:])
                    nc.vector.reciprocal(af[:, 3, :], af[:, 2, :])
                    nc.vector.tensor_tensor(out=af[:, 4, :], in0=gs[:, 0, :],
                                            in1=af[:, 3, :], op=OP.mult)
                    nc.vector.tensor_tensor(out=af[:, 5, :], in0=af[:, 0, :],
                                            in1=af[:, 4, :], op=OP.mult)
                    nc.vector.tensor_tensor(out=af[:, 5, :], in0=gs[:, 1, :],
                                            in1=af[:, 5, :], op=OP.subtract)

                    # ---- apply affine + relu (first halves, then second) ----
                    for hf in range(2):
                        hsl = slice(hf * 2000, (hf + 1) * 2000)
                        for co in range(nco):
                            CO = min(128, cout - co * 128)
                            nc.vector.tensor_scalar(
                                zt[co][0:CO, hsl], zt[co][0:CO, hsl],
                                af[0:CO, 4, co:co + 1], af[0:CO, 5, co:co + 1],
                                OP.mult, OP.add)
                            nc.vector.tensor_scalar_max(
                                zt[co][0:CO, hsl], zt[co][0:CO, hsl], 0.0)
                    if it + 1 < niter:
                        nc.sync.dma_start(gs_sb[l][:], gs_d[it + 1][l][:])
                    rhs = zt

                if nlayers < 8:
                    continue

                # ---------- iteration boundary ----------
                p = it % 2
                if do_knn:
                    # term2: sum_q sw_q |x_new|^2 via (x*sqrt(sw))^2
                    nc.sync.dma_start(q16[:], sws3_d[:])
                    nc.vector.tensor_tensor(out=q16[:], in0=x_cur[:],
                                            in1=q16[:], op=OP.mult)
                    nc.vector.scalar_tensor_tensor(
                        jkV[0:3, :2000], q16[:, 0:2000], 1.0, q16[:, 0:2000],
                        OP.mult, OP.mult,
                        accum_out=a3col[0:3, 2 * it:2 * it + 1])
                    nc.vector.scalar_tensor_tensor(
                        jkS[0:3, :2000], q16[:, 2000:], 1.0, q16[:, 2000:],
                        OP.mult, OP.mult,
                        accum_out=a3col[0:3, 2 * it + 1:2 * it + 2])
                if it + 1 < niter:
                    # refs for next iteration
                    for b in range(BL):
                        nc.sync.dma_start(r_sb[(it + 1) % 2][b][:],
                                          r_d[it + 1][b][:])
                if do_knn:
                    # Le for THIS iteration's e-pass (runs during next iter /
                    # tail), Ld for next iteration's argmax pass
                    build_q(Le, -1.0)
                    if it + 1 < niter:
                        build_q(LdP[(it + 1) % 2], +1.0)

                if do_knn and it == niter - 1:
                    # tail: extraction units for the last iteration
                    vg = False
                    for b in range(BL):
                        for qt in range(8):
                            knn_unit_AB(it, b, qt, vg); vg = not vg

                if do_knn:
                    # loss contribution of iter `it` is complete once its 32
                    # AB units have run: iters 0..2 complete during iter it+1
                    # windows; emit their licol accum at the NEXT boundary (or
                    # now for the final iteration).
                    done_it = it if it == niter - 1 else (it - 1 if it > 0
                                                         else None)
                    for d in ([done_it] if done_it is not None else []):
                        jk2 = sb.tile([128, 32], F32, tag="jk2", bufs=2)
                        nc.vector.scalar_tensor_tensor(
                            jk2[:], SP[d % 2][:], 1.0, sw_sb[:],
                            OP.mult, OP.mult,
                            accum_out=licol[:, d:d + 1])
                    if it == niter - 1 and niter >= 2:
                        # iter (niter-2)'s units finished during iter niter-1
                        d = niter - 2
                        jk2 = sb.tile([128, 32], F32, tag="jk2", bufs=2)
                        nc.vector.scalar_tensor_tensor(
                            jk2[:], SP[d % 2][:], 1.0, sw_sb[:],
                            OP.mult, OP.mult,
                            accum_out=licol[:, d:d + 1])

            nc.sync.dma_start(dbgx_d[:], x_cur[:])
            if nlayers >= 8 and do_knn:
                nc.sync.dma_start(dbgs_d[:], SP[(niter - 1) % 2][:])
                nc.sync.dma_start(dbgm_d[:], mP[(niter - 1) % 2][:])
            lacc = sb.tile([128, 1], F32, tag="laccA")
            nc.vector.tensor_reduce(lacc[:], licol[:], AX.X, OP.add)
            acc3f = sb.tile([3, 1], F32, tag="acc3A")
            nc.vector.tensor_reduce(acc3f[:], a3col[0:3, :], AX.X, OP.add)
            nc.sync.dma_start(loss_d[:], lacc[:])
            nc.sync.dma_start(loss3_d[:], acc3f[:])
    nc.compile()
    return nc


def _host_prep(inputs):
    """Build per-core input maps."""
    f32 = np.float32
    noisy = np.asarray(inputs["pcl_noisy"], f32)
    clean = np.asarray(inputs["pcl_clean"], f32)
    seeds = np.asarray(inputs["pcl_seeds"], f32)
    std = np.asarray(inputs["pcl_std"], f32)
    noise = np.asarray(inputs["noise"], f32)

    pn = noisy - seeds
    pc = clean - seeds
    sdist = np.sum(pn.astype(np.float64) ** 2, -1, keepdims=True)
    max_sq = sdist[:, -1:, :]
    sw = np.exp(-sdist * 9.0 / max_sq)[..., 0]
    sw = (sw / sw.sum(1, keepdims=True))  # [B, N] float64

    tgts = []
    cur = std.copy()
    for i in range(NITER):
        if i < NITER - 1:
            cur = cur / NOISE_DECAY
            tgts.append(pc + noise[i] * cur[:, None, None])
        else:
            tgts.append(pc.copy())

    shared = {}
    for i in range(NITER):
        for l in range(8):
            key = f'ew{l+1}' if l < 5 else f'dw{l-4}'
            w = np.asarray(inputs[key], f32)[i].astype(np.float16)
            cin, cout = w.shape
            nci, nco, cop = NCI[l], NCO[l], COP[l]
            arr = np.zeros((128, nci, nco, cop), np.float16)
            for ci in range(nci):
                cip = min(128, cin - ci * 128)
                for co in range(nco):
                    cw = min(cop, cout - co * cop)
                    arr[0:cip, ci, co, 0:cw] = w[ci * 128:ci * 128 + cip,
                                                 co * cop:co * cop + cw]
            shared[f"w_{i}_{l}"] = arr
        for l in range(7):
            nco = NCO[l]
            cout = LAYERS[l][1]
            gk = f'eg{l+1}' if l < 5 else f'dg{l-4}'
            hk = f'eh{l+1}' if l < 5 else f'dh{l-4}'
            g = np.asarray(inputs[gk], f32)[i]
            h = np.asarray(inputs[hk], f32)[i]
            arr = np.zeros((128, 2, nco), f32)
            gp = np.zeros(nco * 128, f32); gp[:cout] = g
            hp = np.zeros(nco * 128, f32); hp[:cout] = h
            arr[:, 0, :] = gp.reshape(nco, 128).T
            arr[:, 1, :] = hp.reshape(nco, 128).T
            shared[f"gs_{i}_{l}"] = arr
    shared["db3t"] = np.ascontiguousarray(np.asarray(inputs["db3"], f32).T)

    in_maps = []
    for c in range(NCORES):
        bs = slice(c * BL, (c + 1) * BL)
        m = dict(shared)
        m["x0t"] = np.ascontiguousarray(
            pn[bs].transpose(2, 0, 1).reshape(3, PTS))
        swc = np.zeros((128, 32), f32)
        for b in range(BL):
            for qt in range(8):
                swc[0:QT, b * 8 + qt] = sw[c * BL + b,
                                           qt * QT:(qt + 1) * QT].astype(f32)
        m["sw"] = swc
        m["sws3"] = np.broadcast_to(
            np.sqrt(sw[bs]).reshape(1, PTS), (3, PTS)).astype(np.float16)
        for i in range(NITER):
            for b in range(BL):
                coords = np.concatenate(
                    [tgts[i][c * BL + b].astype(np.float64),
                     np.full((RP - N, 3), 100.0, np.float64)], 0)
                rh = coords.astype(np.float16)
                rl = (coords - rh.astype(np.float64)).astype(np.float16)
                rsq = (coords ** 2).sum(1)
                rsqh = rsq.astype(np.float16)
                rsql = (rsq - rsqh.astype(np.float64)).astype(np.float16)
                R = np.empty((11, RP), np.float16)
                R[0:3] = rh.T
                R[3:6] = rh.T
                R[6:9] = rl.T
                R[9] = rsqh
                R[10] = rsql
                m[f"rknn_{i}_{b}"] = R
        in_maps.append(m)
    return in_maps


def kernel(**inputs):
    if "nc" not in _NC_CACHE:
        _NC_CACHE["nc"] = _build()
    nc = _NC_CACHE["nc"]
    in_maps = _host_prep(inputs)
    res = run_bass_kernel_spmd(nc, in_maps, list(range(NCORES))).results
    total = 0.0
    for c in range(NCORES):
        total += float(res[c]["loss_part"].sum())
        total += float(res[c]["loss_part3"].sum())
    return np.asarray(total / B, dtype=np.float32)
